# revision 23
# baseline (speedup 1.0000x reference)
"""Trainium2 Bass kernel for nn_DeBERTaV3CrossAttention.

Strategy (pure data parallel, 16 samples per core on 8 cores):
  - X (last_hidden) uploaded bf16; per-sample on-chip transpose via PE
    matmul-with-identity (also yields masked-sum asp_query via a shared
    N=1 matmul against the 0/1 aspect-mask column).
  - Single-query attention never materializes K/V:
      scores = X @ (wk_h.T q_h)  (bf16 PE matmul, f32 PSUM)
      softmax in f32 (ACT Exp with fused row-sum)
      ctx = (attn @ X) @ wv      (pooled-then-project)
  - All small projections (q, qk, ctx, cross, heads) batched across
    samples in f32.
Host side only prepares masks/layout/padding from sep1/sep2 and
re-packs weights; all tensor compute involving X runs on device.
"""
import sys
sys.path.insert(0, '/opt/trn_rl_repo')
import numpy as np
import ml_dtypes

import concourse.bacc as bacc
import concourse.mybir as mybir
from concourse.tile import TileContext
from concourse.bass_utils import run_bass_kernel_spmd

BF16 = ml_dtypes.bfloat16
F32 = mybir.dt.float32
DTX = mybir.dt.bfloat16
AF = mybir.ActivationFunctionType
AX = mybir.AxisListType.X

B, S, H, NH, DH, NF = 128, 1024, 768, 8, 96, 8
FUSED = 3 * H + NF  # 2312
NCORE = 8
PER = B // NCORE    # 16 samples per core
G = 4               # group size for batched small matmuls
NG = PER // G
NHC = H // 128      # 6 h-chunks
NST = S // 128      # 8 s-tiles
NFC = 19            # ceil(2312/128) fused chunks

_NC = {}
DEBUG_TAPS = False


def _build(n1s, n2s, a0s):
    nc = bacc.Bacc("TRN2", target_bir_lowering=False, debug=False)
    dp = nc.declare_dram_parameter

    xhi = dp("xhi", [PER, S, H], DTX, isOutput=False)
    aspc = dp("aspc", [PER, 128, NST], DTX, isOutput=False)
    maskr = dp("maskr", [PER, NH, S], F32, isOutput=False)
    clsT = dp("clsT", [NHC, 128, PER], DTX, isOutput=False)
    invc = dp("invc", [128, PER], F32, isOutput=False)
    arofd = dp("arof", [PER, NF], F32, isOutput=False)
    lngd = dp("lng", [PER, NF], F32, isOutput=False)
    lnbd = dp("lnb", [PER, NF], F32, isOutput=False)
    i128fd = dp("i128f", [128, 128], F32, isOutput=False)
    wqsTd = dp("wqsT", [128, NHC, 1024], DTX, isOutput=False)
    bqsd = dp("bqs", [128, NH], F32, isOutput=False)
    wkhd = dp("wkh", [128, NH, NHC, 128], DTX, isOutput=False)
    wvTd = dp("wvT", [128, NHC, 1024], DTX, isOutput=False)
    bvpd = dp("bvp", [128, NH], F32, isOutput=False)
    owTd = dp("owT", [128, NH, H], DTX, isOutput=False)
    outbd = dp("outb", [128, NHC], F32, isOutput=False)
    w1Td = dp("w1T", [128, NFC, 512], DTX, isOutput=False)
    b1d = dp("b1", [128, 4], F32, isOutput=False)
    w2Td = dp("w2T", [128, 4, 128], DTX, isOutput=False)
    b2d = dp("b2", [128, 1], F32, isOutput=False)
    w3Td = dp("w3T", [128, 2], DTX, isOutput=False)
    b3d = dp("b3", [128, 1], F32, isOutput=False)
    pwTd = dp("pwT", [128, NFC, 3], DTX, isOutput=False)
    pbd = dp("pb", [128, 1], F32, isOutput=False)
    awTd = dp("awT", [128, NFC, 3], DTX, isOutput=False)
    abd = dp("ab", [128, 1], F32, isOutput=False)

    va_o = dp("va_t", [2, PER], F32, isOutput=True)
    pol_o = dp("pol_t", [3, PER], F32, isOutput=True)
    aro_o = dp("aro_t", [3, PER], F32, isOutput=True)
    if DEBUG_TAPS:
        ts_d = dp("ts_d", [PER, NH, S], F32, isOutput=True)
        e_d = dp("e_d", [PER, NH, S], F32, isOutput=True)
        ppn_d = dp("ppn_d", [PER, NH, H], F32, isOutput=True)
        zz_d = dp("zz_d", [PER, NH], F32, isOutput=True)
        pp_d = dp("pp_d", [PER, NH, H], F32, isOutput=True)
        qk_d = dp("qk_d", [NG, 128, NHC, G, NH], F32, isOutput=True)
        fused_d = dp("fused_d", [128, NFC, PER], DTX, isOutput=True)
        pooled_d = dp("pooled_d", [128, NHC, NH, PER], DTX, isOutput=True)
        ctx_d = dp("ctx_d", [128, NH, PER], DTX, isOutput=True)
        g1_d = dp("g1_d", [128, 4, PER], DTX, isOutput=True)
        g2_d = dp("g2_d", [128, PER], DTX, isOutput=True)

    with TileContext(nc) as tc:
        with (
            tc.tile_pool(name="wpool", bufs=1) as wpool,
            tc.tile_pool(name="spool", bufs=1) as spool,
            tc.tile_pool(name="xpool", bufs=G) as xpool,
            tc.tile_pool(name="xtpool", bufs=6 * G) as xtpool,
            tc.tile_pool(name="wstr", bufs=4) as wstr,
            tc.tile_pool(name="mpool", bufs=2) as mpool,
            tc.tile_pool(name="apool", bufs=3) as apool,
            tc.tile_pool(name="qpool", bufs=2) as qpool,
            tc.tile_pool(name="pepool", bufs=3) as pepool,
            tc.tile_pool(name="smpool", bufs=2) as smpool,
            tc.tile_pool(name="psB", bufs=3, space="PSUM") as psB,
            tc.tile_pool(name="psC", bufs=2, space="PSUM") as psC,
        ):
            # ---- static tiles ----
            i128f = wpool.tile([128, 128], F32)
            nc.sync.dma_start(out=i128f[:, :], in_=i128fd[:, :])
            wq_sb = wpool.tile([128, NHC, 1024], DTX)
            bqs_sb = wpool.tile([128, NH], F32)
            wkh_sb = wpool.tile([128, NH, NHC, 128], DTX)
            invc_sb = wpool.tile([128, PER], F32)
            nc.sync.dma_start(out=invc_sb[:, :], in_=invc[:, :])

            fusedT = spool.tile([128, NFC, PER], DTX)
            pooledT = spool.tile([128, NHC, NH, PER], DTX)
            qkT = spool.tile([128, NHC, G, NH], DTX)

            xts = [None] * PER
            xtTs = [None] * PER

            for g in range(NG):
                # ---- phase A: per-sample load + transpose + asp sums ----
                for l in range(G):
                    i = g * G + l
                    n1, n2, a0 = n1s[i], n2s[i], a0s[i]
                    xt = xpool.tile([128, n2, H], DTX, tag="x")
                    nc.sync.dma_start(
                        out=xt[:, :, :],
                        in_=xhi[i].rearrange("(st p) h -> p st h", p=128)[:, 0:n2, :],
                    )
                    ac = apool.tile([128, NST], DTX, tag="aspc")
                    nc.sync.dma_start(out=ac[:, :], in_=aspc[i])
                    xtT = []
                    for c in range(NHC):
                        t = xtpool.tile([128, n1 * 128], DTX, tag="xt")
                        nc.sync.dma_start_transpose(
                            out=t[:, :],
                            in_=xhi[i][0:n1 * 128, c * 128:(c + 1) * 128])
                        xtT.append(t)
                    xts[i] = xt
                    xtTs[i] = xtT
                    asp6 = psC.tile([128, NHC], F32, tag="small")
                    for c in range(NHC):
                        for st in range(a0, n2):
                            nc.tensor.matmul(
                                asp6[:, c:c + 1],
                                xt[:, st, c * 128:(c + 1) * 128],
                                ac[:, st:st + 1],
                                start=(st == a0), stop=(st == n2 - 1))
                    nc.vector.tensor_scalar(
                        fusedT[:, 12:18, i:i + 1].rearrange("p c one -> p (c one)"),
                        asp6[:, :], invc_sb[:, i:i + 1], None,
                        op0=mybir.AluOpType.mult)

                # ---- phase B: batched q / qk for the group ----
                if g == 0:
                    nc.sync.dma_start(out=wq_sb[:, :, :], in_=wqsTd[:, :, :])
                    nc.sync.dma_start(out=bqs_sb[:, :], in_=bqsd[:, :])
                    nc.sync.dma_start(
                        out=wkh_sb[:, :, :, :], in_=wkhd[:, :, :, :])
                qTt = qpool.tile([128, NH, G], DTX, tag="qT")
                for oc in range(NH):
                    qp = psC.tile([128, G], F32, tag="small")
                    for ic in range(NHC):
                        nc.tensor.matmul(
                            qp[:, :], wq_sb[:, ic, oc * 128:(oc + 1) * 128],
                            fusedT[:, 12 + ic, g * G:(g + 1) * G],
                            start=(ic == 0), stop=(ic == NHC - 1))
                    nc.scalar.activation(
                        qTt[:, oc, :], qp[:, :], AF.Identity,
                        bias=bqs_sb[:, oc:oc + 1])
                for jc in range(NHC):
                    qkp = psC.tile([128, NH * G], F32, tag="small")
                    for h in range(NH):
                        nc.tensor.matmul(
                            qkp[:, h * G:(h + 1) * G], wkh_sb[:, h, jc, :],
                            qTt[:, h, :], start=True, stop=True)
                    nc.any.tensor_copy(
                        qkT[:, jc, :, :],
                        qkp.rearrange("p (h b) -> p b h", h=NH))
                    if DEBUG_TAPS:
                        qkf = smpool.tile([128, G, NH], F32, tag="qkf")
                        nc.any.tensor_copy(
                            qkf[:, :, :],
                            qkp.rearrange("p (h b) -> p b h", h=NH))
                        nc.sync.dma_start(out=qk_d[g, :, jc, :, :],
                                          in_=qkf[:, :, :])

                # ---- phase C: per-sample scores / softmax / pooled ----
                for l in range(G):
                    i = g * G + l
                    n1 = n1s[i]
                    ncol = n1 * 128
                    xt, xtT = xts[i], xtTs[i]
                    sc = psB.tile([NH, ncol], F32, tag="big")
                    for jc in range(NHC):
                        for off in range(0, ncol, 512):
                            w = min(512, ncol - off)
                            nc.tensor.matmul(
                                sc[:, off:off + w],
                                qkT[:, jc, l, :],
                                xtT[jc][:, off:off + w],
                                start=(jc == 0), stop=(jc == NHC - 1))
                    msk = mpool.tile([NH, ncol], F32, tag="msk")
                    nc.sync.dma_start(out=msk[:, :], in_=maskr[i][:, 0:ncol])
                    ts_ = smpool.tile([NH, ncol], F32, tag="ts")
                    nc.vector.tensor_add(ts_[:, :], sc[:, :], msk[:, :])
                    negm = smpool.tile([NH, 1], F32, tag="negm")
                    nc.vector.tensor_reduce(
                        negm[:, :], ts_[:, :], axis=AX,
                        op=mybir.AluOpType.max, negate=True)
                    e = smpool.tile([NH, ncol], F32, tag="e")
                    zz = smpool.tile([NH, 1], F32, tag="z")
                    nc.scalar.activation(
                        e[:, :], ts_[:, :], AF.Exp, bias=negm[:, :],
                        scale=1.0, accum_out=zz[:, :])
                    r = smpool.tile([NH, 1], F32, tag="r")
                    nc.vector.reciprocal(r[:, :], zz[:, :])
                    if DEBUG_TAPS:
                        nc.sync.dma_start(out=ts_d[i], in_=ts_[:, :])
                        nc.sync.dma_start(out=e_d[i], in_=e[:, :])
                    pp = psB.tile([NH, H], F32, tag="big")
                    for st in range(n1):
                        etp = psC.tile([128, NH], F32, tag="small")
                        nc.tensor.matmul(
                            etp[:, :], e[:, st * 128:(st + 1) * 128],
                            i128f[0:NH, 0:NH], start=True, stop=True)
                        peT = pepool.tile([128, NH], DTX, tag="peT")
                        nc.any.tensor_copy(peT[:, :], etp[:, :])
                        for off, n in ((0, 512), (512, 256)):
                            nc.tensor.matmul(
                                pp[:, off:off + n], peT[:, :],
                                xt[:, st, off:off + n],
                                start=(st == 0), stop=(st == n1 - 1))
                    ppn = smpool.tile([NH, H], F32, tag="ppn")
                    nc.vector.tensor_scalar(
                        ppn[:, :], pp[:, :], r[:, :], None,
                        op0=mybir.AluOpType.mult)
                    if DEBUG_TAPS:
                        nc.sync.dma_start(out=ppn_d[i], in_=ppn[:, :])
                        nc.sync.dma_start(out=zz_d[i], in_=zz[:, :])
                        ppf = smpool.tile([NH, H], F32, tag="ppf")
                        nc.vector.tensor_copy(ppf[:, :], pp[:, :])
                        nc.sync.dma_start(out=pp_d[i], in_=ppf[:, :])
                    for jc in range(NHC):
                        ptp = psC.tile([128, NH], F32, tag="small")
                        nc.tensor.matmul(
                            ptp[:, :], ppn[:, jc * 128:(jc + 1) * 128],
                            i128f[0:NH, 0:NH], start=True, stop=True)
                        nc.any.tensor_copy(pooledT[:, jc, :, i], ptp[:, :])
                    xts[i] = None
                    xtTs[i] = None

            # ---- phase D: batched epilogue (weights streamed) ----
            nc.vector.memset(fusedT[:, 18, :], 0.0)
            nc.sync.dma_start(
                out=fusedT[:, 0:NHC, :],
                in_=clsT.rearrange("t p i -> p t i"),
            )
            bvp_sb = wpool.tile([128, NH], F32)
            nc.sync.dma_start(out=bvp_sb[:, :], in_=bvpd[:, :])
            outb_sb = wpool.tile([128, NHC], F32)
            nc.sync.dma_start(out=outb_sb[:, :], in_=outbd[:, :])
            b1_sb = wpool.tile([128, 4], F32)
            nc.sync.dma_start(out=b1_sb[:, :], in_=b1d[:, :])
            b2_sb = wpool.tile([128, 1], F32)
            nc.sync.dma_start(out=b2_sb[:, :], in_=b2d[:, :])
            w3_sb = wpool.tile([128, 2], DTX)
            nc.sync.dma_start(out=w3_sb[:, :], in_=w3Td[:, :])
            b3_sb = wpool.tile([128, 1], F32)
            nc.sync.dma_start(out=b3_sb[:, :], in_=b3d[:, :])
            pb_sb = wpool.tile([128, 1], F32)
            nc.sync.dma_start(out=pb_sb[:, :], in_=pbd[:, :])
            ab_sb = wpool.tile([128, 1], F32)
            nc.sync.dma_start(out=ab_sb[:, :], in_=abd[:, :])

            # ctx: per head  [128(d-pad), PER]
            ctxT = spool.tile([128, NH, PER], DTX)
            for h in range(NH):
                wv_s = wstr.tile([128, NHC, 128], DTX, tag="wst")
                nc.sync.dma_start(
                    out=wv_s[:, :, :], in_=wvTd[:, :, h * 128:(h + 1) * 128])
                cxp = psC.tile([128, PER], F32, tag="small")
                for jc in range(NHC):
                    nc.tensor.matmul(
                        cxp[:, :], wv_s[:, jc, :],
                        pooledT[:, jc, h, :],
                        start=(jc == 0), stop=(jc == NHC - 1))
                nc.scalar.activation(
                    ctxT[:, h, :], cxp[:, :], AF.Identity,
                    bias=bvp_sb[:, h:h + 1])

            # cross -> fusedT[:, 6:12, :]
            for oc in range(NHC):
                ow_s = wstr.tile([128, NH, 128], DTX, tag="wst")
                nc.sync.dma_start(
                    out=ow_s[:, :, :], in_=owTd[:, :, oc * 128:(oc + 1) * 128])
                crp = psC.tile([128, PER], F32, tag="small")
                for ic in range(NH):
                    nc.tensor.matmul(
                        crp[:, :], ow_s[:, ic, :],
                        ctxT[:, ic, :], start=(ic == 0), stop=(ic == NH - 1))
                nc.scalar.activation(
                    fusedT[:, 6 + oc, :], crp[:, :], AF.Identity,
                    bias=outb_sb[:, oc:oc + 1])

            # LayerNorm on arousal feats -> fusedT[0:8, 18, :]
            aro_sb = spool.tile([PER, NF], F32)
            nc.sync.dma_start(out=aro_sb[:, :], in_=arofd[:, :])
            lng_sb = spool.tile([PER, NF], F32)
            nc.sync.dma_start(out=lng_sb[:, :], in_=lngd[:, :])
            lnb_sb = spool.tile([PER, NF], F32)
            nc.sync.dma_start(out=lnb_sb[:, :], in_=lnbd[:, :])
            mu = spool.tile([PER, 1], F32)
            nc.vector.reduce_sum(mu[:, :], aro_sb[:, :], axis=AX)
            nc.vector.tensor_scalar(
                mu[:, :], mu[:, :], 1.0 / NF, None, op0=mybir.AluOpType.mult)
            dd = spool.tile([PER, NF], F32)
            nc.vector.tensor_scalar(
                dd[:, :], aro_sb[:, :], mu[:, :], None,
                op0=mybir.AluOpType.subtract)
            sq = spool.tile([PER, NF], F32)
            ssq = spool.tile([PER, 1], F32)
            nc.scalar.activation(
                sq[:, :], dd[:, :], AF.Square, accum_out=ssq[:, :])
            eps_sb = spool.tile([PER, 1], F32)
            nc.vector.memset(eps_sb[:, :], 1e-5)
            sd = spool.tile([PER, 1], F32)
            nc.scalar.activation(
                sd[:, :], ssq[:, :], AF.Sqrt, scale=1.0 / NF,
                bias=eps_sb[:, :])
            rstd = spool.tile([PER, 1], F32)
            nc.vector.reciprocal(rstd[:, :], sd[:, :])
            an = spool.tile([PER, NF], F32)
            nc.vector.tensor_scalar(
                an[:, :], dd[:, :], rstd[:, :], None,
                op0=mybir.AluOpType.mult)
            nc.vector.tensor_mul(an[:, :], an[:, :], lng_sb[:, :])
            nc.vector.tensor_add(an[:, :], an[:, :], lnb_sb[:, :])
            trp = psC.tile([NF, PER], F32, tag="small")
            nc.tensor.matmul(
                trp[:, :], an[:, :], i128f[0:PER, 0:PER],
                start=True, stop=True)
            nc.any.tensor_copy(fusedT[0:NF, 18, :], trp[:, :])

            # heads
            g1 = spool.tile([128, 4, PER], DTX)
            for oc in range(4):
                w1_s = wstr.tile([128, NFC, 128], DTX, tag="wst")
                nc.sync.dma_start(
                    out=w1_s[:, :, :], in_=w1Td[:, :, oc * 128:(oc + 1) * 128])
                hp = psC.tile([128, PER], F32, tag="small")
                for fc in range(NFC):
                    nc.tensor.matmul(
                        hp[:, :], w1_s[:, fc, :],
                        fusedT[:, fc, :], start=(fc == 0), stop=(fc == NFC - 1))
                nc.scalar.activation(
                    g1[:, oc, :], hp[:, :], AF.Gelu, bias=b1_sb[:, oc:oc + 1])
            g2 = spool.tile([128, PER], DTX)
            w2_s = wstr.tile([128, 4, 128], DTX, tag="wst")
            nc.sync.dma_start(out=w2_s[:, :, :], in_=w2Td[:, :, :])
            hp2 = psC.tile([128, PER], F32, tag="small")
            for fc in range(4):
                nc.tensor.matmul(
                    hp2[:, :], w2_s[:, fc, :], g1[:, fc, :],
                    start=(fc == 0), stop=(fc == 3))
            nc.scalar.activation(
                g2[:, :], hp2[:, :], AF.Gelu, bias=b2_sb[:, 0:1])
            vap = psC.tile([2, PER], F32, tag="small")
            nc.tensor.matmul(
                vap[:, :], w3_sb[:, :], g2[:, :], start=True, stop=True)
            va_sb = spool.tile([2, PER], F32)
            nc.scalar.activation(
                va_sb[:, :], vap[:, :], AF.Identity, bias=b3_sb[0:2, 0:1])
            nc.sync.dma_start(out=va_o[:, :], in_=va_sb[:, :])

            for wTd_, bb, oo in ((pwTd, pb_sb, pol_o), (awTd, ab_sb, aro_o)):
                w_s = wstr.tile([128, NFC, 3], DTX, tag="wst")
                nc.sync.dma_start(out=w_s[:, :, :], in_=wTd_[:, :, :])
                p3 = psC.tile([3, PER], F32, tag="small")
                for fc in range(NFC):
                    nc.tensor.matmul(
                        p3[:, :], w_s[:, fc, :], fusedT[:, fc, :],
                        start=(fc == 0), stop=(fc == NFC - 1))
                o_sb = spool.tile([3, PER], F32, tag="osb")
                nc.scalar.activation(
                    o_sb[:, :], p3[:, :], AF.Identity, bias=bb[0:3, 0:1])
                nc.sync.dma_start(out=oo[:, :], in_=o_sb[:, :])
            if DEBUG_TAPS:
                nc.sync.dma_start(out=fused_d[:, :, :], in_=fusedT[:, :, :])
                nc.sync.dma_start(out=pooled_d[:, :, :, :], in_=pooledT[:, :, :, :])
                nc.sync.dma_start(out=ctx_d[:, :, :], in_=ctxT[:, :, :])
                nc.sync.dma_start(out=g1_d[:, :, :], in_=g1[:, :, :])
                nc.sync.dma_start(out=g2_d[:, :], in_=g2[:, :])

    nc.compile()
    return nc


def _get_nc(n1s, n2s, a0s):
    key = (tuple(n1s), tuple(n2s), tuple(a0s))
    if key not in _NC:
        _NC[key] = _build(tuple(n1s), tuple(n2s), tuple(a0s))
    return _NC[key]


def _pad_heads(w):
    """[NH*DH, X] -> [128, NH, X] zero-padded along d (96->128)."""
    x = w.reshape(NH, DH, -1)
    out = np.zeros((NH, 128, x.shape[2]), np.float32)
    out[:, :DH, :] = x
    return out.transpose(1, 0, 2)  # [128, NH, X]


def _prep_shared(inputs):
    ipw = np.asarray(inputs["in_proj_w"], np.float32)
    ipb = np.asarray(inputs["in_proj_b"], np.float32)
    out_w = np.asarray(inputs["out_w"], np.float32)
    out_b = np.asarray(inputs["out_b"], np.float32)
    wq, wk, wv = ipw[0:H], ipw[H:2 * H], ipw[2 * H:3 * H]
    bq, bv = ipb[0:H], ipb[2 * H:3 * H]
    scale = 1.0 / np.sqrt(DH)

    d = {}
    # wqsT [128, 6, 1024]: [i_loc, ic, h*128+dpad] = wq[h*96+d, ic*128+i]*scale
    t = _pad_heads(wq * scale)            # [128(dpad), NH, H(i)]
    arr = t.transpose(2, 1, 0).reshape(H, NH * 128)  # [i, opad]
    d["wqsT"] = np.ascontiguousarray(
        arr.reshape(NHC, 128, NH * 128).transpose(1, 0, 2)).astype(BF16)
    z = np.zeros((NH, 128), np.float32)
    z[:, :DH] = (bq * scale).reshape(NH, DH)
    d["bqs"] = np.ascontiguousarray(z.T)  # [128, NH]
    # wkh [128(dpad), NH, NHC, 128]: wk[h*96+d, jc*128+j]
    t = _pad_heads(wk)                    # [128, NH, H(j)]
    d["wkh"] = np.ascontiguousarray(t.reshape(128, NH, NHC, 128)).astype(BF16)
    # wvT [128(j_loc), NHC, 1024(h*128+dpad)]
    t = _pad_heads(wv)                    # [128(dpad), NH, H(j)]
    arr = t.transpose(2, 1, 0).reshape(H, NH * 128)  # [j, opad]
    d["wvT"] = np.ascontiguousarray(
        arr.reshape(NHC, 128, NH * 128).transpose(1, 0, 2)).astype(BF16)
    z = np.zeros((NH, 128), np.float32)
    z[:, :DH] = bv.reshape(NH, DH)
    d["bvp"] = np.ascontiguousarray(z.T)
    # owT [128(dpad), NH(ic), 768(o)] = out_w[o, h*96+d]
    t = _pad_heads(out_w.T.copy().reshape(NH * DH, H))
    d["owT"] = np.ascontiguousarray(t).astype(BF16)
    d["outb"] = np.ascontiguousarray(out_b.reshape(NHC, 128).T)
    # heads
    w1 = np.asarray(inputs["va_w1"], np.float32)
    t = np.zeros((NFC * 128, 512), np.float32)
    t[:FUSED] = w1.T
    d["w1T"] = np.ascontiguousarray(t.reshape(NFC, 128, 512).transpose(1, 0, 2)).astype(BF16)
    d["b1"] = np.ascontiguousarray(
        np.asarray(inputs["va_b1"], np.float32).reshape(4, 128).T)
    w2 = np.asarray(inputs["va_w2"], np.float32)
    d["w2T"] = np.ascontiguousarray(w2.T.reshape(4, 128, 128).transpose(1, 0, 2)).astype(BF16)
    d["b2"] = np.asarray(inputs["va_b2"], np.float32).reshape(128, 1)
    d["w3T"] = np.ascontiguousarray(np.asarray(inputs["va_w3"], np.float32).T).astype(BF16)
    z = np.zeros((128, 1), np.float32)
    z[:2, 0] = np.asarray(inputs["va_b3"], np.float32)
    d["b3"] = z
    for wname, bname, wkey, bkey in (
            ("pol_w", "pol_b", "pwT", "pb"), ("aro_w", "aro_b", "awT", "ab")):
        wx = np.asarray(inputs[wname], np.float32)
        t = np.zeros((NFC * 128, 3), np.float32)
        t[:FUSED] = wx.T
        d[wkey] = np.ascontiguousarray(t.reshape(NFC, 128, 3).transpose(1, 0, 2)).astype(BF16)
        z = np.zeros((128, 1), np.float32)
        z[:3, 0] = np.asarray(inputs[bname], np.float32)
        d[bkey] = z
    d["i128b"] = np.eye(128, dtype=BF16)
    d["i128f"] = np.eye(128, dtype=np.float32)
    return d


def _prepare_in_maps(inputs):
    lh = np.asarray(inputs["last_hidden"], np.float32)
    arf = np.asarray(inputs["arousal_feats"], np.float32)
    sep1 = np.asarray(inputs["sep1"]).astype(np.int64)
    sep2 = np.asarray(inputs["sep2"]).astype(np.int64)
    ln_g = np.asarray(inputs["ln_g"], np.float32)
    ln_b = np.asarray(inputs["ln_b"], np.float32)

    shared = _prep_shared(inputs)

    pos = np.arange(S)
    tm = np.where((sep1 > 1)[:, None],
                  (pos[None, :] >= 1) & (pos[None, :] < sep1[:, None]),
                  pos[None, :] == 0)
    am = np.where((sep2 > sep1 + 1)[:, None],
                  (pos[None, :] > sep1[:, None]) & (pos[None, :] < sep2[:, None]),
                  pos[None, :] == 0)
    maskadd = np.where(tm, 0.0, -1e30).astype(np.float32)
    cnt = am.sum(1).astype(np.float32)

    # ragged tile bounds per sample
    n1_all = np.where(sep1 > 1, np.ceil(sep1 / 128), 1).astype(np.int64)
    n1_all = np.maximum(n1_all, 1)
    n2a = np.where(sep2 > sep1 + 1, np.ceil(sep2 / 128), 1).astype(np.int64)
    n2_all = np.maximum(n1_all, np.maximum(n2a, 1))
    # sort samples by size desc, deal round-robin to cores so slot j is
    # similar across cores; per-slot bound = max over cores
    order = np.argsort(-(n2_all * 16 + n1_all), kind="stable")
    perm = np.empty(B, np.int64)
    for c in range(NCORE):
        for j in range(PER):
            perm[c * PER + j] = order[j * NCORE + c]
    n1s = tuple(int(max(n1_all[perm[c * PER + j]] for c in range(NCORE)))
                for j in range(PER))
    n2s = tuple(int(max(max(n2_all[perm[c * PER + j]], n1_all[perm[c * PER + j]])
                        for c in range(NCORE))) for j in range(PER))
    n2s = tuple(max(a, b) for a, b in zip(n1s, n2s))
    # first tile containing any aspect-mask weight (fallback case -> tile 0)
    a0_all = np.where(sep2 > sep1 + 1, sep1 // 128, 0).astype(np.int64)
    a0s = tuple(int(min(a0_all[perm[c * PER + j]] for c in range(NCORE)))
                for j in range(PER))
    a0s = tuple(min(a, b - 1) for a, b in zip(a0s, n2s))

    in_maps = []
    for c in range(NCORE):
        idx = perm[c * PER:(c + 1) * PER]
        m = dict(shared)
        m["xhi"] = lh[idx].astype(BF16)
        m["aspc"] = np.ascontiguousarray(
            am[idx].astype(np.float32).astype(BF16)
            .reshape(PER, NST, 128).transpose(0, 2, 1))
        m["maskr"] = np.ascontiguousarray(
            np.repeat(maskadd[idx][:, None, :], NH, axis=1))
        m["clsT"] = np.ascontiguousarray(
            lh[idx, 0, :].T.reshape(NHC, 128, PER)).astype(BF16)
        m["invc"] = np.tile((1.0 / cnt[idx])[None, :], (128, 1)).astype(np.float32)
        m["arof"] = np.ascontiguousarray(arf[idx])
        m["lng"] = np.tile(ln_g[None, :], (PER, 1)).astype(np.float32)
        m["lnb"] = np.tile(ln_b[None, :], (PER, 1)).astype(np.float32)
        in_maps.append(m)
    return in_maps, perm, n1s, n2s, a0s


def _assemble(results, perm):
    va = np.zeros((B, 2), np.float32)
    pol = np.zeros((B, 3), np.float32)
    aro = np.zeros((B, 3), np.float32)
    for c in range(NCORE):
        idx = perm[c * PER:(c + 1) * PER]
        va[idx] = results[c]["va_t"].T
        pol[idx] = results[c]["pol_t"].T
        aro[idx] = results[c]["aro_t"].T
    return (va, pol, aro)


def kernel(**inputs):
    in_maps, perm, n1s, n2s, a0s = _prepare_in_maps(inputs)
    nc = _get_nc(n1s, n2s, a0s)
    res = run_bass_kernel_spmd(nc, in_maps, core_ids=list(range(NCORE)))
    return _assemble(res.results, perm)


# revision 24
# speedup vs baseline: 1.2583x; 1.2583x over previous
"""Trainium2 Bass kernel for nn_DeBERTaV3CrossAttention.

Strategy (pure data parallel, 16 samples per core on 8 cores):
  - X (last_hidden) uploaded bf16; per-sample on-chip transpose via PE
    matmul-with-identity (also yields masked-sum asp_query via a shared
    N=1 matmul against the 0/1 aspect-mask column).
  - Single-query attention never materializes K/V:
      scores = X @ (wk_h.T q_h)  (bf16 PE matmul, f32 PSUM)
      softmax in f32 (ACT Exp with fused row-sum)
      ctx = (attn @ X) @ wv      (pooled-then-project)
  - All small projections (q, qk, ctx, cross, heads) batched across
    samples in f32.
Host side only prepares masks/layout/padding from sep1/sep2 and
re-packs weights; all tensor compute involving X runs on device.
"""
import sys
sys.path.insert(0, '/opt/trn_rl_repo')
import numpy as np
import ml_dtypes

import concourse.bacc as bacc
import concourse.mybir as mybir
from concourse.tile import TileContext
from concourse.bass_utils import run_bass_kernel_spmd

BF16 = ml_dtypes.bfloat16
F32 = mybir.dt.float32
DTX = mybir.dt.bfloat16
AF = mybir.ActivationFunctionType
AX = mybir.AxisListType.X

B, S, H, NH, DH, NF = 128, 1024, 768, 8, 96, 8
FUSED = 3 * H + NF  # 2312
NCORE = 8
PER = B // NCORE    # 16 samples per core
G = 4               # group size for batched small matmuls
NG = PER // G
NHC = H // 128      # 6 h-chunks
NST = S // 128      # 8 s-tiles
NFC = 19            # ceil(2312/128) fused chunks

_NC = {}
DEBUG_TAPS = False


def _build(n1s, n2s, a0s):
    nc = bacc.Bacc("TRN2", target_bir_lowering=False, debug=False)
    dp = nc.declare_dram_parameter

    xhi = dp("xhi", [PER, S, H], DTX, isOutput=False)
    aspc = dp("aspc", [PER, 128, NST], DTX, isOutput=False)
    maskr = dp("maskr", [PER, NH, S], F32, isOutput=False)
    clsT = dp("clsT", [NHC, 128, PER], DTX, isOutput=False)
    invc = dp("invc", [128, PER], F32, isOutput=False)
    arofd = dp("arof", [PER, NF], F32, isOutput=False)
    lngd = dp("lng", [PER, NF], F32, isOutput=False)
    lnbd = dp("lnb", [PER, NF], F32, isOutput=False)
    i128fd = dp("i128f", [128, 128], F32, isOutput=False)
    wqsTd = dp("wqsT", [128, NHC, 1024], DTX, isOutput=False)
    bqsd = dp("bqs", [128, NH], F32, isOutput=False)
    wkhd = dp("wkh", [128, NH, NHC, 128], DTX, isOutput=False)
    wvTd = dp("wvT", [128, NHC, 1024], DTX, isOutput=False)
    bvpd = dp("bvp", [128, NH], F32, isOutput=False)
    owTd = dp("owT", [128, NH, H], DTX, isOutput=False)
    outbd = dp("outb", [128, NHC], F32, isOutput=False)
    w1Td = dp("w1T", [128, NFC, 512], DTX, isOutput=False)
    b1d = dp("b1", [128, 4], F32, isOutput=False)
    w2Td = dp("w2T", [128, 4, 128], DTX, isOutput=False)
    b2d = dp("b2", [128, 1], F32, isOutput=False)
    w3Td = dp("w3T", [128, 2], DTX, isOutput=False)
    b3d = dp("b3", [128, 1], F32, isOutput=False)
    pwTd = dp("pwT", [128, NFC, 3], DTX, isOutput=False)
    pbd = dp("pb", [128, 1], F32, isOutput=False)
    awTd = dp("awT", [128, NFC, 3], DTX, isOutput=False)
    abd = dp("ab", [128, 1], F32, isOutput=False)

    va_o = dp("va_t", [2, PER], F32, isOutput=True)
    pol_o = dp("pol_t", [3, PER], F32, isOutput=True)
    aro_o = dp("aro_t", [3, PER], F32, isOutput=True)
    if DEBUG_TAPS:
        ts_d = dp("ts_d", [PER, NH, S], F32, isOutput=True)
        e_d = dp("e_d", [PER, NH, S], F32, isOutput=True)
        ppn_d = dp("ppn_d", [PER, NH, H], F32, isOutput=True)
        zz_d = dp("zz_d", [PER, NH], F32, isOutput=True)
        pp_d = dp("pp_d", [PER, NH, H], F32, isOutput=True)
        qk_d = dp("qk_d", [NG, 128, NHC, G, NH], F32, isOutput=True)
        fused_d = dp("fused_d", [128, NFC, PER], DTX, isOutput=True)
        pooled_d = dp("pooled_d", [128, NHC, NH, PER], DTX, isOutput=True)
        ctx_d = dp("ctx_d", [128, NH, PER], DTX, isOutput=True)
        g1_d = dp("g1_d", [128, 4, PER], DTX, isOutput=True)
        g2_d = dp("g2_d", [128, PER], DTX, isOutput=True)

    with TileContext(nc) as tc:
        with (
            tc.tile_pool(name="wpool", bufs=1) as wpool,
            tc.tile_pool(name="spool", bufs=1) as spool,
            tc.tile_pool(name="xpool", bufs=G) as xpool,
            tc.tile_pool(name="xtpool", bufs=18) as xtpool,
            tc.tile_pool(name="wstr", bufs=4) as wstr,
            tc.tile_pool(name="mpool", bufs=2) as mpool,
            tc.tile_pool(name="apool", bufs=3) as apool,
            tc.tile_pool(name="qpool", bufs=2) as qpool,
            tc.tile_pool(name="pepool", bufs=3) as pepool,
            tc.tile_pool(name="smpool", bufs=2) as smpool,
            tc.tile_pool(name="psB", bufs=3, space="PSUM") as psB,
            tc.tile_pool(name="psC", bufs=2, space="PSUM") as psC,
        ):
            # ---- static tiles ----
            i128f = wpool.tile([128, 128], F32)
            nc.sync.dma_start(out=i128f[:, :], in_=i128fd[:, :])
            wq_sb = wpool.tile([128, NHC, 1024], DTX)
            bqs_sb = wpool.tile([128, NH], F32)
            wkh_sb = wpool.tile([128, NH, NHC, 128], DTX)
            invc_sb = wpool.tile([128, PER], F32)
            nc.sync.dma_start(out=invc_sb[:, :], in_=invc[:, :])

            fusedT = spool.tile([128, NFC, PER], DTX)
            pooledT = spool.tile([128, NHC, NH, PER], DTX)
            qkT = spool.tile([128, NHC, G, NH], DTX)

            xts = [None] * PER
            xtTs = [None] * PER

            for g in range(NG):
                # ---- phase A: per-sample load + transpose + asp sums ----
                for l in range(G):
                    i = g * G + l
                    n1, n2, a0 = n1s[i], n2s[i], a0s[i]
                    xt = xpool.tile([128, n2, H], DTX, tag="x")
                    nc.sync.dma_start(
                        out=xt[:, :, :],
                        in_=xhi[i].rearrange("(st p) h -> p st h", p=128)[:, 0:n2, :],
                    )
                    ac = apool.tile([128, NST], DTX, tag="aspc")
                    nc.sync.dma_start(out=ac[:, :], in_=aspc[i])
                    xts[i] = xt
                    asp6 = psC.tile([128, NHC], F32, tag="small")
                    for c in range(NHC):
                        for st in range(a0, n2):
                            nc.tensor.matmul(
                                asp6[:, c:c + 1],
                                xt[:, st, c * 128:(c + 1) * 128],
                                ac[:, st:st + 1],
                                start=(st == a0), stop=(st == n2 - 1))
                    nc.vector.tensor_scalar(
                        fusedT[:, 12:18, i:i + 1].rearrange("p c one -> p (c one)"),
                        asp6[:, :], invc_sb[:, i:i + 1], None,
                        op0=mybir.AluOpType.mult)

                # ---- phase B: batched q / qk for the group ----
                if g == 0:
                    nc.sync.dma_start(out=wq_sb[:, :, :], in_=wqsTd[:, :, :])
                    nc.sync.dma_start(out=bqs_sb[:, :], in_=bqsd[:, :])
                    nc.sync.dma_start(
                        out=wkh_sb[:, :, :, :], in_=wkhd[:, :, :, :])
                qTt = qpool.tile([128, NH, G], DTX, tag="qT")
                for oc in range(NH):
                    qp = psC.tile([128, G], F32, tag="small")
                    for ic in range(NHC):
                        nc.tensor.matmul(
                            qp[:, :], wq_sb[:, ic, oc * 128:(oc + 1) * 128],
                            fusedT[:, 12 + ic, g * G:(g + 1) * G],
                            start=(ic == 0), stop=(ic == NHC - 1))
                    nc.scalar.activation(
                        qTt[:, oc, :], qp[:, :], AF.Identity,
                        bias=bqs_sb[:, oc:oc + 1])
                for jc in range(NHC):
                    qkp = psC.tile([128, NH * G], F32, tag="small")
                    for h in range(NH):
                        nc.tensor.matmul(
                            qkp[:, h * G:(h + 1) * G], wkh_sb[:, h, jc, :],
                            qTt[:, h, :], start=True, stop=True)
                    nc.any.tensor_copy(
                        qkT[:, jc, :, :],
                        qkp.rearrange("p (h b) -> p b h", h=NH))
                    if DEBUG_TAPS:
                        qkf = smpool.tile([128, G, NH], F32, tag="qkf")
                        nc.any.tensor_copy(
                            qkf[:, :, :],
                            qkp.rearrange("p (h b) -> p b h", h=NH))
                        nc.sync.dma_start(out=qk_d[g, :, jc, :, :],
                                          in_=qkf[:, :, :])

                # ---- phase C: per-sample scores / softmax / pooled ----
                for l in range(G):
                    i = g * G + l
                    n1 = n1s[i]
                    ncol = n1 * 128
                    xt = xts[i]
                    xtT = []
                    for c in range(NHC):
                        t = xtpool.tile([128, ncol], DTX, tag="xt")
                        nc.sync.dma_start_transpose(
                            out=t[:, :],
                            in_=xhi[i][0:ncol, c * 128:(c + 1) * 128])
                        xtT.append(t)
                    sc = psB.tile([NH, ncol], F32, tag="big")
                    for jc in range(NHC):
                        for off in range(0, ncol, 512):
                            w = min(512, ncol - off)
                            nc.tensor.matmul(
                                sc[:, off:off + w],
                                qkT[:, jc, l, :],
                                xtT[jc][:, off:off + w],
                                start=(jc == 0), stop=(jc == NHC - 1))
                    msk = mpool.tile([NH, ncol], F32, tag="msk")
                    nc.sync.dma_start(out=msk[:, :], in_=maskr[i][:, 0:ncol])
                    ts_ = smpool.tile([NH, ncol], F32, tag="ts")
                    nc.vector.tensor_add(ts_[:, :], sc[:, :], msk[:, :])
                    negm = smpool.tile([NH, 1], F32, tag="negm")
                    nc.vector.tensor_reduce(
                        negm[:, :], ts_[:, :], axis=AX,
                        op=mybir.AluOpType.max, negate=True)
                    e = smpool.tile([NH, ncol], F32, tag="e")
                    zz = smpool.tile([NH, 1], F32, tag="z")
                    nc.scalar.activation(
                        e[:, :], ts_[:, :], AF.Exp, bias=negm[:, :],
                        scale=1.0, accum_out=zz[:, :])
                    r = smpool.tile([NH, 1], F32, tag="r")
                    nc.vector.reciprocal(r[:, :], zz[:, :])
                    if DEBUG_TAPS:
                        nc.sync.dma_start(out=ts_d[i], in_=ts_[:, :])
                        nc.sync.dma_start(out=e_d[i], in_=e[:, :])
                    pp = psB.tile([NH, H], F32, tag="big")
                    for st in range(n1):
                        etp = psC.tile([128, NH], F32, tag="small")
                        nc.tensor.matmul(
                            etp[:, :], e[:, st * 128:(st + 1) * 128],
                            i128f[0:NH, 0:NH], start=True, stop=True)
                        peT = pepool.tile([128, NH], DTX, tag="peT")
                        nc.any.tensor_copy(peT[:, :], etp[:, :])
                        for off, n in ((0, 512), (512, 256)):
                            nc.tensor.matmul(
                                pp[:, off:off + n], peT[:, :],
                                xt[:, st, off:off + n],
                                start=(st == 0), stop=(st == n1 - 1))
                    ppn = smpool.tile([NH, H], F32, tag="ppn")
                    nc.vector.tensor_scalar(
                        ppn[:, :], pp[:, :], r[:, :], None,
                        op0=mybir.AluOpType.mult)
                    if DEBUG_TAPS:
                        nc.sync.dma_start(out=ppn_d[i], in_=ppn[:, :])
                        nc.sync.dma_start(out=zz_d[i], in_=zz[:, :])
                        ppf = smpool.tile([NH, H], F32, tag="ppf")
                        nc.vector.tensor_copy(ppf[:, :], pp[:, :])
                        nc.sync.dma_start(out=pp_d[i], in_=ppf[:, :])
                    for jc in range(NHC):
                        ptp = psC.tile([128, NH], F32, tag="small")
                        nc.tensor.matmul(
                            ptp[:, :], ppn[:, jc * 128:(jc + 1) * 128],
                            i128f[0:NH, 0:NH], start=True, stop=True)
                        nc.any.tensor_copy(pooledT[:, jc, :, i], ptp[:, :])
                    xts[i] = None

            # ---- phase D: batched epilogue (weights streamed) ----
            nc.vector.memset(fusedT[:, 18, :], 0.0)
            nc.sync.dma_start(
                out=fusedT[:, 0:NHC, :],
                in_=clsT.rearrange("t p i -> p t i"),
            )
            bvp_sb = wpool.tile([128, NH], F32)
            nc.sync.dma_start(out=bvp_sb[:, :], in_=bvpd[:, :])
            outb_sb = wpool.tile([128, NHC], F32)
            nc.sync.dma_start(out=outb_sb[:, :], in_=outbd[:, :])
            b1_sb = wpool.tile([128, 4], F32)
            nc.sync.dma_start(out=b1_sb[:, :], in_=b1d[:, :])
            b2_sb = wpool.tile([128, 1], F32)
            nc.sync.dma_start(out=b2_sb[:, :], in_=b2d[:, :])
            w3_sb = wpool.tile([128, 2], DTX)
            nc.sync.dma_start(out=w3_sb[:, :], in_=w3Td[:, :])
            b3_sb = wpool.tile([128, 1], F32)
            nc.sync.dma_start(out=b3_sb[:, :], in_=b3d[:, :])
            pb_sb = wpool.tile([128, 1], F32)
            nc.sync.dma_start(out=pb_sb[:, :], in_=pbd[:, :])
            ab_sb = wpool.tile([128, 1], F32)
            nc.sync.dma_start(out=ab_sb[:, :], in_=abd[:, :])

            # ctx: per head  [128(d-pad), PER]
            ctxT = spool.tile([128, NH, PER], DTX)
            for h in range(NH):
                wv_s = wstr.tile([128, NHC, 128], DTX, tag="wst")
                nc.sync.dma_start(
                    out=wv_s[:, :, :], in_=wvTd[:, :, h * 128:(h + 1) * 128])
                cxp = psC.tile([128, PER], F32, tag="small")
                for jc in range(NHC):
                    nc.tensor.matmul(
                        cxp[:, :], wv_s[:, jc, :],
                        pooledT[:, jc, h, :],
                        start=(jc == 0), stop=(jc == NHC - 1))
                nc.scalar.activation(
                    ctxT[:, h, :], cxp[:, :], AF.Identity,
                    bias=bvp_sb[:, h:h + 1])

            # cross -> fusedT[:, 6:12, :]
            for oc in range(NHC):
                ow_s = wstr.tile([128, NH, 128], DTX, tag="wst")
                nc.sync.dma_start(
                    out=ow_s[:, :, :], in_=owTd[:, :, oc * 128:(oc + 1) * 128])
                crp = psC.tile([128, PER], F32, tag="small")
                for ic in range(NH):
                    nc.tensor.matmul(
                        crp[:, :], ow_s[:, ic, :],
                        ctxT[:, ic, :], start=(ic == 0), stop=(ic == NH - 1))
                nc.scalar.activation(
                    fusedT[:, 6 + oc, :], crp[:, :], AF.Identity,
                    bias=outb_sb[:, oc:oc + 1])

            # LayerNorm on arousal feats -> fusedT[0:8, 18, :]
            aro_sb = spool.tile([PER, NF], F32)
            nc.sync.dma_start(out=aro_sb[:, :], in_=arofd[:, :])
            lng_sb = spool.tile([PER, NF], F32)
            nc.sync.dma_start(out=lng_sb[:, :], in_=lngd[:, :])
            lnb_sb = spool.tile([PER, NF], F32)
            nc.sync.dma_start(out=lnb_sb[:, :], in_=lnbd[:, :])
            mu = spool.tile([PER, 1], F32)
            nc.vector.reduce_sum(mu[:, :], aro_sb[:, :], axis=AX)
            nc.vector.tensor_scalar(
                mu[:, :], mu[:, :], 1.0 / NF, None, op0=mybir.AluOpType.mult)
            dd = spool.tile([PER, NF], F32)
            nc.vector.tensor_scalar(
                dd[:, :], aro_sb[:, :], mu[:, :], None,
                op0=mybir.AluOpType.subtract)
            sq = spool.tile([PER, NF], F32)
            ssq = spool.tile([PER, 1], F32)
            nc.scalar.activation(
                sq[:, :], dd[:, :], AF.Square, accum_out=ssq[:, :])
            eps_sb = spool.tile([PER, 1], F32)
            nc.vector.memset(eps_sb[:, :], 1e-5)
            sd = spool.tile([PER, 1], F32)
            nc.scalar.activation(
                sd[:, :], ssq[:, :], AF.Sqrt, scale=1.0 / NF,
                bias=eps_sb[:, :])
            rstd = spool.tile([PER, 1], F32)
            nc.vector.reciprocal(rstd[:, :], sd[:, :])
            an = spool.tile([PER, NF], F32)
            nc.vector.tensor_scalar(
                an[:, :], dd[:, :], rstd[:, :], None,
                op0=mybir.AluOpType.mult)
            nc.vector.tensor_mul(an[:, :], an[:, :], lng_sb[:, :])
            nc.vector.tensor_add(an[:, :], an[:, :], lnb_sb[:, :])
            trp = psC.tile([NF, PER], F32, tag="small")
            nc.tensor.matmul(
                trp[:, :], an[:, :], i128f[0:PER, 0:PER],
                start=True, stop=True)
            nc.any.tensor_copy(fusedT[0:NF, 18, :], trp[:, :])

            # heads
            g1 = spool.tile([128, 4, PER], DTX)
            for oc in range(4):
                w1_s = wstr.tile([128, NFC, 128], DTX, tag="wst")
                nc.sync.dma_start(
                    out=w1_s[:, :, :], in_=w1Td[:, :, oc * 128:(oc + 1) * 128])
                hp = psC.tile([128, PER], F32, tag="small")
                for fc in range(NFC):
                    nc.tensor.matmul(
                        hp[:, :], w1_s[:, fc, :],
                        fusedT[:, fc, :], start=(fc == 0), stop=(fc == NFC - 1))
                nc.scalar.activation(
                    g1[:, oc, :], hp[:, :], AF.Gelu, bias=b1_sb[:, oc:oc + 1])
            g2 = spool.tile([128, PER], DTX)
            w2_s = wstr.tile([128, 4, 128], DTX, tag="wst")
            nc.sync.dma_start(out=w2_s[:, :, :], in_=w2Td[:, :, :])
            hp2 = psC.tile([128, PER], F32, tag="small")
            for fc in range(4):
                nc.tensor.matmul(
                    hp2[:, :], w2_s[:, fc, :], g1[:, fc, :],
                    start=(fc == 0), stop=(fc == 3))
            nc.scalar.activation(
                g2[:, :], hp2[:, :], AF.Gelu, bias=b2_sb[:, 0:1])
            vap = psC.tile([2, PER], F32, tag="small")
            nc.tensor.matmul(
                vap[:, :], w3_sb[:, :], g2[:, :], start=True, stop=True)
            va_sb = spool.tile([2, PER], F32)
            nc.scalar.activation(
                va_sb[:, :], vap[:, :], AF.Identity, bias=b3_sb[0:2, 0:1])
            nc.sync.dma_start(out=va_o[:, :], in_=va_sb[:, :])

            for wTd_, bb, oo in ((pwTd, pb_sb, pol_o), (awTd, ab_sb, aro_o)):
                w_s = wstr.tile([128, NFC, 3], DTX, tag="wst")
                nc.sync.dma_start(out=w_s[:, :, :], in_=wTd_[:, :, :])
                p3 = psC.tile([3, PER], F32, tag="small")
                for fc in range(NFC):
                    nc.tensor.matmul(
                        p3[:, :], w_s[:, fc, :], fusedT[:, fc, :],
                        start=(fc == 0), stop=(fc == NFC - 1))
                o_sb = spool.tile([3, PER], F32, tag="osb")
                nc.scalar.activation(
                    o_sb[:, :], p3[:, :], AF.Identity, bias=bb[0:3, 0:1])
                nc.sync.dma_start(out=oo[:, :], in_=o_sb[:, :])
            if DEBUG_TAPS:
                nc.sync.dma_start(out=fused_d[:, :, :], in_=fusedT[:, :, :])
                nc.sync.dma_start(out=pooled_d[:, :, :, :], in_=pooledT[:, :, :, :])
                nc.sync.dma_start(out=ctx_d[:, :, :], in_=ctxT[:, :, :])
                nc.sync.dma_start(out=g1_d[:, :, :], in_=g1[:, :, :])
                nc.sync.dma_start(out=g2_d[:, :], in_=g2[:, :])

    nc.compile()
    return nc


def _get_nc(n1s, n2s, a0s):
    key = (tuple(n1s), tuple(n2s), tuple(a0s))
    if key not in _NC:
        _NC[key] = _build(tuple(n1s), tuple(n2s), tuple(a0s))
    return _NC[key]


def _pad_heads(w):
    """[NH*DH, X] -> [128, NH, X] zero-padded along d (96->128)."""
    x = w.reshape(NH, DH, -1)
    out = np.zeros((NH, 128, x.shape[2]), np.float32)
    out[:, :DH, :] = x
    return out.transpose(1, 0, 2)  # [128, NH, X]


def _prep_shared(inputs):
    ipw = np.asarray(inputs["in_proj_w"], np.float32)
    ipb = np.asarray(inputs["in_proj_b"], np.float32)
    out_w = np.asarray(inputs["out_w"], np.float32)
    out_b = np.asarray(inputs["out_b"], np.float32)
    wq, wk, wv = ipw[0:H], ipw[H:2 * H], ipw[2 * H:3 * H]
    bq, bv = ipb[0:H], ipb[2 * H:3 * H]
    scale = 1.0 / np.sqrt(DH)

    d = {}
    # wqsT [128, 6, 1024]: [i_loc, ic, h*128+dpad] = wq[h*96+d, ic*128+i]*scale
    t = _pad_heads(wq * scale)            # [128(dpad), NH, H(i)]
    arr = t.transpose(2, 1, 0).reshape(H, NH * 128)  # [i, opad]
    d["wqsT"] = np.ascontiguousarray(
        arr.reshape(NHC, 128, NH * 128).transpose(1, 0, 2)).astype(BF16)
    z = np.zeros((NH, 128), np.float32)
    z[:, :DH] = (bq * scale).reshape(NH, DH)
    d["bqs"] = np.ascontiguousarray(z.T)  # [128, NH]
    # wkh [128(dpad), NH, NHC, 128]: wk[h*96+d, jc*128+j]
    t = _pad_heads(wk)                    # [128, NH, H(j)]
    d["wkh"] = np.ascontiguousarray(t.reshape(128, NH, NHC, 128)).astype(BF16)
    # wvT [128(j_loc), NHC, 1024(h*128+dpad)]
    t = _pad_heads(wv)                    # [128(dpad), NH, H(j)]
    arr = t.transpose(2, 1, 0).reshape(H, NH * 128)  # [j, opad]
    d["wvT"] = np.ascontiguousarray(
        arr.reshape(NHC, 128, NH * 128).transpose(1, 0, 2)).astype(BF16)
    z = np.zeros((NH, 128), np.float32)
    z[:, :DH] = bv.reshape(NH, DH)
    d["bvp"] = np.ascontiguousarray(z.T)
    # owT [128(dpad), NH(ic), 768(o)] = out_w[o, h*96+d]
    t = _pad_heads(out_w.T.copy().reshape(NH * DH, H))
    d["owT"] = np.ascontiguousarray(t).astype(BF16)
    d["outb"] = np.ascontiguousarray(out_b.reshape(NHC, 128).T)
    # heads
    w1 = np.asarray(inputs["va_w1"], np.float32)
    t = np.zeros((NFC * 128, 512), np.float32)
    t[:FUSED] = w1.T
    d["w1T"] = np.ascontiguousarray(t.reshape(NFC, 128, 512).transpose(1, 0, 2)).astype(BF16)
    d["b1"] = np.ascontiguousarray(
        np.asarray(inputs["va_b1"], np.float32).reshape(4, 128).T)
    w2 = np.asarray(inputs["va_w2"], np.float32)
    d["w2T"] = np.ascontiguousarray(w2.T.reshape(4, 128, 128).transpose(1, 0, 2)).astype(BF16)
    d["b2"] = np.asarray(inputs["va_b2"], np.float32).reshape(128, 1)
    d["w3T"] = np.ascontiguousarray(np.asarray(inputs["va_w3"], np.float32).T).astype(BF16)
    z = np.zeros((128, 1), np.float32)
    z[:2, 0] = np.asarray(inputs["va_b3"], np.float32)
    d["b3"] = z
    for wname, bname, wkey, bkey in (
            ("pol_w", "pol_b", "pwT", "pb"), ("aro_w", "aro_b", "awT", "ab")):
        wx = np.asarray(inputs[wname], np.float32)
        t = np.zeros((NFC * 128, 3), np.float32)
        t[:FUSED] = wx.T
        d[wkey] = np.ascontiguousarray(t.reshape(NFC, 128, 3).transpose(1, 0, 2)).astype(BF16)
        z = np.zeros((128, 1), np.float32)
        z[:3, 0] = np.asarray(inputs[bname], np.float32)
        d[bkey] = z
    d["i128b"] = np.eye(128, dtype=BF16)
    d["i128f"] = np.eye(128, dtype=np.float32)
    return d


def _prepare_in_maps(inputs):
    lh = np.asarray(inputs["last_hidden"], np.float32)
    arf = np.asarray(inputs["arousal_feats"], np.float32)
    sep1 = np.asarray(inputs["sep1"]).astype(np.int64)
    sep2 = np.asarray(inputs["sep2"]).astype(np.int64)
    ln_g = np.asarray(inputs["ln_g"], np.float32)
    ln_b = np.asarray(inputs["ln_b"], np.float32)

    shared = _prep_shared(inputs)

    pos = np.arange(S)
    tm = np.where((sep1 > 1)[:, None],
                  (pos[None, :] >= 1) & (pos[None, :] < sep1[:, None]),
                  pos[None, :] == 0)
    am = np.where((sep2 > sep1 + 1)[:, None],
                  (pos[None, :] > sep1[:, None]) & (pos[None, :] < sep2[:, None]),
                  pos[None, :] == 0)
    maskadd = np.where(tm, 0.0, -1e30).astype(np.float32)
    cnt = am.sum(1).astype(np.float32)

    # ragged tile bounds per sample
    n1_all = np.where(sep1 > 1, np.ceil(sep1 / 128), 1).astype(np.int64)
    n1_all = np.maximum(n1_all, 1)
    n2a = np.where(sep2 > sep1 + 1, np.ceil(sep2 / 128), 1).astype(np.int64)
    n2_all = np.maximum(n1_all, np.maximum(n2a, 1))
    # sort samples by size desc, deal round-robin to cores so slot j is
    # similar across cores; per-slot bound = max over cores
    order = np.argsort(-(n2_all * 16 + n1_all), kind="stable")
    perm = np.empty(B, np.int64)
    for c in range(NCORE):
        for j in range(PER):
            perm[c * PER + j] = order[j * NCORE + c]
    n1s = tuple(int(max(n1_all[perm[c * PER + j]] for c in range(NCORE)))
                for j in range(PER))
    n2s = tuple(int(max(max(n2_all[perm[c * PER + j]], n1_all[perm[c * PER + j]])
                        for c in range(NCORE))) for j in range(PER))
    n2s = tuple(max(a, b) for a, b in zip(n1s, n2s))
    # first tile containing any aspect-mask weight (fallback case -> tile 0)
    a0_all = np.where(sep2 > sep1 + 1, sep1 // 128, 0).astype(np.int64)
    a0s = tuple(int(min(a0_all[perm[c * PER + j]] for c in range(NCORE)))
                for j in range(PER))
    a0s = tuple(min(a, b - 1) for a, b in zip(a0s, n2s))

    in_maps = []
    for c in range(NCORE):
        idx = perm[c * PER:(c + 1) * PER]
        m = dict(shared)
        m["xhi"] = lh[idx].astype(BF16)
        m["aspc"] = np.ascontiguousarray(
            am[idx].astype(np.float32).astype(BF16)
            .reshape(PER, NST, 128).transpose(0, 2, 1))
        m["maskr"] = np.ascontiguousarray(
            np.repeat(maskadd[idx][:, None, :], NH, axis=1))
        m["clsT"] = np.ascontiguousarray(
            lh[idx, 0, :].T.reshape(NHC, 128, PER)).astype(BF16)
        m["invc"] = np.tile((1.0 / cnt[idx])[None, :], (128, 1)).astype(np.float32)
        m["arof"] = np.ascontiguousarray(arf[idx])
        m["lng"] = np.tile(ln_g[None, :], (PER, 1)).astype(np.float32)
        m["lnb"] = np.tile(ln_b[None, :], (PER, 1)).astype(np.float32)
        in_maps.append(m)
    return in_maps, perm, n1s, n2s, a0s


def _assemble(results, perm):
    va = np.zeros((B, 2), np.float32)
    pol = np.zeros((B, 3), np.float32)
    aro = np.zeros((B, 3), np.float32)
    for c in range(NCORE):
        idx = perm[c * PER:(c + 1) * PER]
        va[idx] = results[c]["va_t"].T
        pol[idx] = results[c]["pol_t"].T
        aro[idx] = results[c]["aro_t"].T
    return (va, pol, aro)


def kernel(**inputs):
    in_maps, perm, n1s, n2s, a0s = _prepare_in_maps(inputs)
    nc = _get_nc(n1s, n2s, a0s)
    res = run_bass_kernel_spmd(nc, in_maps, core_ids=list(range(NCORE)))
    return _assemble(res.results, perm)


# revision 26
# speedup vs baseline: 2.0001x; 1.5895x over previous
"""Trainium2 Bass kernel for nn_DeBERTaV3CrossAttention.

Strategy (pure data parallel, 16 samples per core on 8 cores):
  - X (last_hidden) uploaded bf16; per-sample on-chip transpose via PE
    matmul-with-identity (also yields masked-sum asp_query via a shared
    N=1 matmul against the 0/1 aspect-mask column).
  - Single-query attention never materializes K/V:
      scores = X @ (wk_h.T q_h)  (bf16 PE matmul, f32 PSUM)
      softmax in f32 (ACT Exp with fused row-sum)
      ctx = (attn @ X) @ wv      (pooled-then-project)
  - All small projections (q, qk, ctx, cross, heads) batched across
    samples in f32.
Host side only prepares masks/layout/padding from sep1/sep2 and
re-packs weights; all tensor compute involving X runs on device.
"""
import sys
sys.path.insert(0, '/opt/trn_rl_repo')
import numpy as np
import ml_dtypes

import concourse.bacc as bacc
import concourse.mybir as mybir
from concourse.tile import TileContext
from concourse.bass_utils import run_bass_kernel_spmd

BF16 = ml_dtypes.bfloat16
F32 = mybir.dt.float32
DTX = mybir.dt.bfloat16
AF = mybir.ActivationFunctionType
AX = mybir.AxisListType.X

B, S, H, NH, DH, NF = 128, 1024, 768, 8, 96, 8
FUSED = 3 * H + NF  # 2312
NCORE = 8
PER = B // NCORE    # 16 samples per core
G = 4               # group size for batched small matmuls
NG = PER // G
NHC = H // 128      # 6 h-chunks
NST = S // 128      # 8 s-tiles
NFC = 19            # ceil(2312/128) fused chunks

_NC = {}
DEBUG_TAPS = False


def _build(n1s, n2s, a0s):
    nc = bacc.Bacc("TRN2", target_bir_lowering=False, debug=False)
    dp = nc.declare_dram_parameter

    xhi = dp("xhi", [PER, S, H], DTX, isOutput=False)
    aspc = dp("aspc", [PER, 128, NST], DTX, isOutput=False)
    maskr = dp("maskr", [PER, NH, S], DTX, isOutput=False)
    clsT = dp("clsT", [NHC, 128, PER], DTX, isOutput=False)
    invc = dp("invc", [128, PER], F32, isOutput=False)
    arofd = dp("arof", [PER, NF], F32, isOutput=False)
    lngd = dp("lng", [PER, NF], F32, isOutput=False)
    lnbd = dp("lnb", [PER, NF], F32, isOutput=False)
    i128fd = dp("i128f", [128, 128], F32, isOutput=False)
    wqsTd = dp("wqsT", [128, NHC, 1024], DTX, isOutput=False)
    bqsd = dp("bqs", [128, NH], F32, isOutput=False)
    wkhd = dp("wkh", [128, NH, NHC, 128], DTX, isOutput=False)
    wvTd = dp("wvT", [128, NHC, 1024], DTX, isOutput=False)
    bvpd = dp("bvp", [128, NH], F32, isOutput=False)
    owTd = dp("owT", [128, NH, H], DTX, isOutput=False)
    outbd = dp("outb", [128, NHC], F32, isOutput=False)
    w1Td = dp("w1T", [128, NFC, 512], DTX, isOutput=False)
    b1d = dp("b1", [128, 4], F32, isOutput=False)
    w2Td = dp("w2T", [128, 4, 128], DTX, isOutput=False)
    b2d = dp("b2", [128, 1], F32, isOutput=False)
    w3Td = dp("w3T", [128, 2], DTX, isOutput=False)
    b3d = dp("b3", [128, 1], F32, isOutput=False)
    pwTd = dp("pwT", [128, NFC, 3], DTX, isOutput=False)
    pbd = dp("pb", [128, 1], F32, isOutput=False)
    awTd = dp("awT", [128, NFC, 3], DTX, isOutput=False)
    abd = dp("ab", [128, 1], F32, isOutput=False)

    va_o = dp("va_t", [2, PER], F32, isOutput=True)
    pol_o = dp("pol_t", [3, PER], F32, isOutput=True)
    aro_o = dp("aro_t", [3, PER], F32, isOutput=True)
    if DEBUG_TAPS:
        ts_d = dp("ts_d", [PER, NH, S], F32, isOutput=True)
        e_d = dp("e_d", [PER, NH, S], F32, isOutput=True)
        ppn_d = dp("ppn_d", [PER, NH, H], F32, isOutput=True)
        zz_d = dp("zz_d", [PER, NH], F32, isOutput=True)
        pp_d = dp("pp_d", [PER, NH, H], F32, isOutput=True)
        qk_d = dp("qk_d", [NG, 128, NHC, G, NH], F32, isOutput=True)
        fused_d = dp("fused_d", [128, NFC, PER], DTX, isOutput=True)
        pooled_d = dp("pooled_d", [128, NHC, NH, PER], DTX, isOutput=True)
        ctx_d = dp("ctx_d", [128, NH, PER], DTX, isOutput=True)
        g1_d = dp("g1_d", [128, 4, PER], DTX, isOutput=True)
        g2_d = dp("g2_d", [128, PER], DTX, isOutput=True)

    with TileContext(nc) as tc:
        with (
            tc.tile_pool(name="wpool", bufs=1) as wpool,
            tc.tile_pool(name="spool", bufs=1) as spool,
            tc.tile_pool(name="xpool", bufs=6) as xpool,
            tc.tile_pool(name="xtpool", bufs=18) as xtpool,
            tc.tile_pool(name="wstr", bufs=4) as wstr,
            tc.tile_pool(name="mpool", bufs=2) as mpool,
            tc.tile_pool(name="apool", bufs=3) as apool,
            tc.tile_pool(name="qpool", bufs=2) as qpool,
            tc.tile_pool(name="pepool", bufs=3) as pepool,
            tc.tile_pool(name="smpool", bufs=2) as smpool,
            tc.tile_pool(name="psB", bufs=3, space="PSUM") as psB,
            tc.tile_pool(name="psC", bufs=2, space="PSUM") as psC,
        ):
            # ---- static tiles ----
            i128f = wpool.tile([128, 128], F32)
            nc.sync.dma_start(out=i128f[:, :], in_=i128fd[:, :])
            wq_sb = wpool.tile([128, NHC, 1024], DTX)
            bqs_sb = wpool.tile([128, NH], F32)
            wkh_sb = wpool.tile([128, NH, NHC, 128], DTX)
            invc_sb = wpool.tile([128, PER], F32)
            nc.sync.dma_start(out=invc_sb[:, :], in_=invc[:, :])

            fusedT = spool.tile([128, NFC, PER], DTX)
            pooledT = spool.tile([128, NHC, NH, PER], DTX)

            xts = [None] * PER
            xtTs = [None] * PER

            for g in range(NG):
                # ---- phase A: per-sample load + transpose + asp sums ----
                acg = apool.tile([128, G, NST], DTX, tag="aspc")
                nc.sync.dma_start(
                    out=acg[:, :, :],
                    in_=aspc[g * G:(g + 1) * G].rearrange("l p st -> p l st"))
                mskg = mpool.tile([NH, G, S], DTX, tag="msk")
                nc.sync.dma_start(
                    out=mskg[:, :, :],
                    in_=maskr[g * G:(g + 1) * G].rearrange("l h s -> h l s"))
                for l in range(G):
                    i = g * G + l
                    n1, n2, a0 = n1s[i], n2s[i], a0s[i]
                    xt = xpool.tile([128, n2, H], DTX, tag="x")
                    nc.sync.dma_start(
                        out=xt[:, :, :],
                        in_=xhi[i].rearrange("(st p) h -> p st h", p=128)[:, 0:n2, :],
                    )
                    ac = acg[:, l, :]
                    xts[i] = xt
                    asp6 = psC.tile([128, NHC], F32, tag="small")
                    for c in range(NHC):
                        for st in range(a0, n2):
                            nc.tensor.matmul(
                                asp6[:, c:c + 1],
                                xt[:, st, c * 128:(c + 1) * 128],
                                ac[:, st:st + 1],
                                start=(st == a0), stop=(st == n2 - 1))
                    nc.vector.tensor_scalar(
                        fusedT[:, 12:18, i:i + 1].rearrange("p c one -> p (c one)"),
                        asp6[:, :], invc_sb[:, i:i + 1], None,
                        op0=mybir.AluOpType.mult)

                # ---- phase B: batched q / qk for the group ----
                if g == 0:
                    nc.sync.dma_start(out=wq_sb[:, :, :], in_=wqsTd[:, :, :])
                    nc.sync.dma_start(out=bqs_sb[:, :], in_=bqsd[:, :])
                    nc.sync.dma_start(
                        out=wkh_sb[:, :, :, :], in_=wkhd[:, :, :, :])
                qkT = qpool.tile([128, NHC, G, NH], DTX, tag="qkT")
                qTt = qpool.tile([128, NH, G], DTX, tag="qT")
                for oc in range(NH):
                    qp = psC.tile([128, G], F32, tag="small")
                    for ic in range(NHC):
                        nc.tensor.matmul(
                            qp[:, :], wq_sb[:, ic, oc * 128:(oc + 1) * 128],
                            fusedT[:, 12 + ic, g * G:(g + 1) * G],
                            start=(ic == 0), stop=(ic == NHC - 1))
                    nc.scalar.activation(
                        qTt[:, oc, :], qp[:, :], AF.Identity,
                        bias=bqs_sb[:, oc:oc + 1])
                for jc in range(NHC):
                    qkp = psC.tile([128, NH * G], F32, tag="small")
                    for h in range(NH):
                        nc.tensor.matmul(
                            qkp[:, h * G:(h + 1) * G], wkh_sb[:, h, jc, :],
                            qTt[:, h, :], start=True, stop=True)
                    nc.any.tensor_copy(
                        qkT[:, jc, :, :],
                        qkp.rearrange("p (h b) -> p b h", h=NH))
                    if DEBUG_TAPS:
                        qkf = smpool.tile([128, G, NH], F32, tag="qkf")
                        nc.any.tensor_copy(
                            qkf[:, :, :],
                            qkp.rearrange("p (h b) -> p b h", h=NH))
                        nc.sync.dma_start(out=qk_d[g, :, jc, :, :],
                                          in_=qkf[:, :, :])

                # ---- phase C: per-sample scores / softmax / pooled ----
                for l in range(G):
                    i = g * G + l
                    n1 = n1s[i]
                    ncol = n1 * 128
                    xt = xts[i]
                    xtT = []
                    for c in range(NHC):
                        t = xtpool.tile([128, ncol], DTX, tag="xt")
                        nc.sync.dma_start_transpose(
                            out=t[:, :],
                            in_=xhi[i][0:ncol, c * 128:(c + 1) * 128])
                        xtT.append(t)
                    sc = psB.tile([NH, ncol], F32, tag="big")
                    for jc in range(NHC):
                        for off in range(0, ncol, 512):
                            w = min(512, ncol - off)
                            nc.tensor.matmul(
                                sc[:, off:off + w],
                                qkT[:, jc, l, :],
                                xtT[jc][:, off:off + w],
                                start=(jc == 0), stop=(jc == NHC - 1))
                    ts_ = smpool.tile([NH, ncol], F32, tag="ts")
                    nc.vector.tensor_add(ts_[:, :], sc[:, :],
                                         mskg[:, l, 0:ncol])
                    negm = smpool.tile([NH, 1], F32, tag="negm")
                    nc.vector.tensor_reduce(
                        negm[:, :], ts_[:, :], axis=AX,
                        op=mybir.AluOpType.max, negate=True)
                    e = smpool.tile([NH, ncol], F32, tag="e")
                    zz = smpool.tile([NH, 1], F32, tag="z")
                    nc.scalar.activation(
                        e[:, :], ts_[:, :], AF.Exp, bias=negm[:, :],
                        scale=1.0, accum_out=zz[:, :])
                    r = smpool.tile([NH, 1], F32, tag="r")
                    nc.vector.reciprocal(r[:, :], zz[:, :])
                    if DEBUG_TAPS:
                        nc.sync.dma_start(out=ts_d[i], in_=ts_[:, :])
                        nc.sync.dma_start(out=e_d[i], in_=e[:, :])
                    pp = psB.tile([NH, H], F32, tag="big")
                    for st in range(n1):
                        etp = psC.tile([128, NH], F32, tag="small")
                        nc.tensor.matmul(
                            etp[:, :], e[:, st * 128:(st + 1) * 128],
                            i128f[0:NH, 0:NH], start=True, stop=True)
                        peT = pepool.tile([128, NH], DTX, tag="peT")
                        nc.any.tensor_copy(peT[:, :], etp[:, :])
                        for off, n in ((0, 512), (512, 256)):
                            nc.tensor.matmul(
                                pp[:, off:off + n], peT[:, :],
                                xt[:, st, off:off + n],
                                start=(st == 0), stop=(st == n1 - 1))
                    ppn = smpool.tile([NH, H], F32, tag="ppn")
                    nc.vector.tensor_scalar(
                        ppn[:, :], pp[:, :], r[:, :], None,
                        op0=mybir.AluOpType.mult)
                    if DEBUG_TAPS:
                        nc.sync.dma_start(out=ppn_d[i], in_=ppn[:, :])
                        nc.sync.dma_start(out=zz_d[i], in_=zz[:, :])
                        ppf = smpool.tile([NH, H], F32, tag="ppf")
                        nc.vector.tensor_copy(ppf[:, :], pp[:, :])
                        nc.sync.dma_start(out=pp_d[i], in_=ppf[:, :])
                    for jc in range(NHC):
                        ptp = psC.tile([128, NH], F32, tag="small")
                        nc.tensor.matmul(
                            ptp[:, :], ppn[:, jc * 128:(jc + 1) * 128],
                            i128f[0:NH, 0:NH], start=True, stop=True)
                        nc.any.tensor_copy(pooledT[:, jc, :, i], ptp[:, :])
                    xts[i] = None

            # ---- phase D: batched epilogue (weights streamed) ----
            nc.vector.memset(fusedT[:, 18, :], 0.0)
            nc.sync.dma_start(
                out=fusedT[:, 0:NHC, :],
                in_=clsT.rearrange("t p i -> p t i"),
            )
            bvp_sb = wpool.tile([128, NH], F32)
            nc.sync.dma_start(out=bvp_sb[:, :], in_=bvpd[:, :])
            outb_sb = wpool.tile([128, NHC], F32)
            nc.sync.dma_start(out=outb_sb[:, :], in_=outbd[:, :])
            b1_sb = wpool.tile([128, 4], F32)
            nc.sync.dma_start(out=b1_sb[:, :], in_=b1d[:, :])
            b2_sb = wpool.tile([128, 1], F32)
            nc.sync.dma_start(out=b2_sb[:, :], in_=b2d[:, :])
            w3_sb = wpool.tile([128, 2], DTX)
            nc.sync.dma_start(out=w3_sb[:, :], in_=w3Td[:, :])
            b3_sb = wpool.tile([128, 1], F32)
            nc.sync.dma_start(out=b3_sb[:, :], in_=b3d[:, :])
            pb_sb = wpool.tile([128, 1], F32)
            nc.sync.dma_start(out=pb_sb[:, :], in_=pbd[:, :])
            ab_sb = wpool.tile([128, 1], F32)
            nc.sync.dma_start(out=ab_sb[:, :], in_=abd[:, :])

            # ctx: per head  [128(d-pad), PER]
            ctxT = spool.tile([128, NH, PER], DTX)
            for h in range(NH):
                wv_s = wstr.tile([128, NHC, 128], DTX, tag="wst")
                nc.sync.dma_start(
                    out=wv_s[:, :, :], in_=wvTd[:, :, h * 128:(h + 1) * 128])
                cxp = psC.tile([128, PER], F32, tag="small")
                for jc in range(NHC):
                    nc.tensor.matmul(
                        cxp[:, :], wv_s[:, jc, :],
                        pooledT[:, jc, h, :],
                        start=(jc == 0), stop=(jc == NHC - 1))
                nc.scalar.activation(
                    ctxT[:, h, :], cxp[:, :], AF.Identity,
                    bias=bvp_sb[:, h:h + 1])

            # cross -> fusedT[:, 6:12, :]
            for oc in range(NHC):
                ow_s = wstr.tile([128, NH, 128], DTX, tag="wst")
                nc.sync.dma_start(
                    out=ow_s[:, :, :], in_=owTd[:, :, oc * 128:(oc + 1) * 128])
                crp = psC.tile([128, PER], F32, tag="small")
                for ic in range(NH):
                    nc.tensor.matmul(
                        crp[:, :], ow_s[:, ic, :],
                        ctxT[:, ic, :], start=(ic == 0), stop=(ic == NH - 1))
                nc.scalar.activation(
                    fusedT[:, 6 + oc, :], crp[:, :], AF.Identity,
                    bias=outb_sb[:, oc:oc + 1])

            # LayerNorm on arousal feats -> fusedT[0:8, 18, :]
            aro_sb = spool.tile([PER, NF], F32)
            nc.sync.dma_start(out=aro_sb[:, :], in_=arofd[:, :])
            lng_sb = spool.tile([PER, NF], F32)
            nc.sync.dma_start(out=lng_sb[:, :], in_=lngd[:, :])
            lnb_sb = spool.tile([PER, NF], F32)
            nc.sync.dma_start(out=lnb_sb[:, :], in_=lnbd[:, :])
            mu = spool.tile([PER, 1], F32)
            nc.vector.reduce_sum(mu[:, :], aro_sb[:, :], axis=AX)
            nc.vector.tensor_scalar(
                mu[:, :], mu[:, :], 1.0 / NF, None, op0=mybir.AluOpType.mult)
            dd = spool.tile([PER, NF], F32)
            nc.vector.tensor_scalar(
                dd[:, :], aro_sb[:, :], mu[:, :], None,
                op0=mybir.AluOpType.subtract)
            sq = spool.tile([PER, NF], F32)
            ssq = spool.tile([PER, 1], F32)
            nc.scalar.activation(
                sq[:, :], dd[:, :], AF.Square, accum_out=ssq[:, :])
            eps_sb = spool.tile([PER, 1], F32)
            nc.vector.memset(eps_sb[:, :], 1e-5)
            sd = spool.tile([PER, 1], F32)
            nc.scalar.activation(
                sd[:, :], ssq[:, :], AF.Sqrt, scale=1.0 / NF,
                bias=eps_sb[:, :])
            rstd = spool.tile([PER, 1], F32)
            nc.vector.reciprocal(rstd[:, :], sd[:, :])
            an = spool.tile([PER, NF], F32)
            nc.vector.tensor_scalar(
                an[:, :], dd[:, :], rstd[:, :], None,
                op0=mybir.AluOpType.mult)
            nc.vector.tensor_mul(an[:, :], an[:, :], lng_sb[:, :])
            nc.vector.tensor_add(an[:, :], an[:, :], lnb_sb[:, :])
            trp = psC.tile([NF, PER], F32, tag="small")
            nc.tensor.matmul(
                trp[:, :], an[:, :], i128f[0:PER, 0:PER],
                start=True, stop=True)
            nc.any.tensor_copy(fusedT[0:NF, 18, :], trp[:, :])

            # heads
            g1 = spool.tile([128, 4, PER], DTX)
            for oc in range(4):
                w1_s = wstr.tile([128, NFC, 128], DTX, tag="wst")
                nc.sync.dma_start(
                    out=w1_s[:, :, :], in_=w1Td[:, :, oc * 128:(oc + 1) * 128])
                hp = psC.tile([128, PER], F32, tag="small")
                for fc in range(NFC):
                    nc.tensor.matmul(
                        hp[:, :], w1_s[:, fc, :],
                        fusedT[:, fc, :], start=(fc == 0), stop=(fc == NFC - 1))
                nc.scalar.activation(
                    g1[:, oc, :], hp[:, :], AF.Gelu, bias=b1_sb[:, oc:oc + 1])
            g2 = spool.tile([128, PER], DTX)
            w2_s = wstr.tile([128, 4, 128], DTX, tag="wst")
            nc.sync.dma_start(out=w2_s[:, :, :], in_=w2Td[:, :, :])
            hp2 = psC.tile([128, PER], F32, tag="small")
            for fc in range(4):
                nc.tensor.matmul(
                    hp2[:, :], w2_s[:, fc, :], g1[:, fc, :],
                    start=(fc == 0), stop=(fc == 3))
            nc.scalar.activation(
                g2[:, :], hp2[:, :], AF.Gelu, bias=b2_sb[:, 0:1])
            vap = psC.tile([2, PER], F32, tag="small")
            nc.tensor.matmul(
                vap[:, :], w3_sb[:, :], g2[:, :], start=True, stop=True)
            va_sb = spool.tile([2, PER], F32)
            nc.scalar.activation(
                va_sb[:, :], vap[:, :], AF.Identity, bias=b3_sb[0:2, 0:1])
            nc.sync.dma_start(out=va_o[:, :], in_=va_sb[:, :])

            for wTd_, bb, oo in ((pwTd, pb_sb, pol_o), (awTd, ab_sb, aro_o)):
                w_s = wstr.tile([128, NFC, 3], DTX, tag="wst")
                nc.sync.dma_start(out=w_s[:, :, :], in_=wTd_[:, :, :])
                p3 = psC.tile([3, PER], F32, tag="small")
                for fc in range(NFC):
                    nc.tensor.matmul(
                        p3[:, :], w_s[:, fc, :], fusedT[:, fc, :],
                        start=(fc == 0), stop=(fc == NFC - 1))
                o_sb = spool.tile([3, PER], F32, tag="osb")
                nc.scalar.activation(
                    o_sb[:, :], p3[:, :], AF.Identity, bias=bb[0:3, 0:1])
                nc.sync.dma_start(out=oo[:, :], in_=o_sb[:, :])
            if DEBUG_TAPS:
                nc.sync.dma_start(out=fused_d[:, :, :], in_=fusedT[:, :, :])
                nc.sync.dma_start(out=pooled_d[:, :, :, :], in_=pooledT[:, :, :, :])
                nc.sync.dma_start(out=ctx_d[:, :, :], in_=ctxT[:, :, :])
                nc.sync.dma_start(out=g1_d[:, :, :], in_=g1[:, :, :])
                nc.sync.dma_start(out=g2_d[:, :], in_=g2[:, :])

    nc.compile()
    return nc


def _get_nc(n1s, n2s, a0s):
    key = (tuple(n1s), tuple(n2s), tuple(a0s))
    if key not in _NC:
        _NC[key] = _build(tuple(n1s), tuple(n2s), tuple(a0s))
    return _NC[key]


def _pad_heads(w):
    """[NH*DH, X] -> [128, NH, X] zero-padded along d (96->128)."""
    x = w.reshape(NH, DH, -1)
    out = np.zeros((NH, 128, x.shape[2]), np.float32)
    out[:, :DH, :] = x
    return out.transpose(1, 0, 2)  # [128, NH, X]


def _prep_shared(inputs):
    ipw = np.asarray(inputs["in_proj_w"], np.float32)
    ipb = np.asarray(inputs["in_proj_b"], np.float32)
    out_w = np.asarray(inputs["out_w"], np.float32)
    out_b = np.asarray(inputs["out_b"], np.float32)
    wq, wk, wv = ipw[0:H], ipw[H:2 * H], ipw[2 * H:3 * H]
    bq, bv = ipb[0:H], ipb[2 * H:3 * H]
    scale = 1.0 / np.sqrt(DH)

    d = {}
    # wqsT [128, 6, 1024]: [i_loc, ic, h*128+dpad] = wq[h*96+d, ic*128+i]*scale
    t = _pad_heads(wq * scale)            # [128(dpad), NH, H(i)]
    arr = t.transpose(2, 1, 0).reshape(H, NH * 128)  # [i, opad]
    d["wqsT"] = np.ascontiguousarray(
        arr.reshape(NHC, 128, NH * 128).transpose(1, 0, 2)).astype(BF16)
    z = np.zeros((NH, 128), np.float32)
    z[:, :DH] = (bq * scale).reshape(NH, DH)
    d["bqs"] = np.ascontiguousarray(z.T)  # [128, NH]
    # wkh [128(dpad), NH, NHC, 128]: wk[h*96+d, jc*128+j]
    t = _pad_heads(wk)                    # [128, NH, H(j)]
    d["wkh"] = np.ascontiguousarray(t.reshape(128, NH, NHC, 128)).astype(BF16)
    # wvT [128(j_loc), NHC, 1024(h*128+dpad)]
    t = _pad_heads(wv)                    # [128(dpad), NH, H(j)]
    arr = t.transpose(2, 1, 0).reshape(H, NH * 128)  # [j, opad]
    d["wvT"] = np.ascontiguousarray(
        arr.reshape(NHC, 128, NH * 128).transpose(1, 0, 2)).astype(BF16)
    z = np.zeros((NH, 128), np.float32)
    z[:, :DH] = bv.reshape(NH, DH)
    d["bvp"] = np.ascontiguousarray(z.T)
    # owT [128(dpad), NH(ic), 768(o)] = out_w[o, h*96+d]
    t = _pad_heads(out_w.T.copy().reshape(NH * DH, H))
    d["owT"] = np.ascontiguousarray(t).astype(BF16)
    d["outb"] = np.ascontiguousarray(out_b.reshape(NHC, 128).T)
    # heads
    w1 = np.asarray(inputs["va_w1"], np.float32)
    t = np.zeros((NFC * 128, 512), np.float32)
    t[:FUSED] = w1.T
    d["w1T"] = np.ascontiguousarray(t.reshape(NFC, 128, 512).transpose(1, 0, 2)).astype(BF16)
    d["b1"] = np.ascontiguousarray(
        np.asarray(inputs["va_b1"], np.float32).reshape(4, 128).T)
    w2 = np.asarray(inputs["va_w2"], np.float32)
    d["w2T"] = np.ascontiguousarray(w2.T.reshape(4, 128, 128).transpose(1, 0, 2)).astype(BF16)
    d["b2"] = np.asarray(inputs["va_b2"], np.float32).reshape(128, 1)
    d["w3T"] = np.ascontiguousarray(np.asarray(inputs["va_w3"], np.float32).T).astype(BF16)
    z = np.zeros((128, 1), np.float32)
    z[:2, 0] = np.asarray(inputs["va_b3"], np.float32)
    d["b3"] = z
    for wname, bname, wkey, bkey in (
            ("pol_w", "pol_b", "pwT", "pb"), ("aro_w", "aro_b", "awT", "ab")):
        wx = np.asarray(inputs[wname], np.float32)
        t = np.zeros((NFC * 128, 3), np.float32)
        t[:FUSED] = wx.T
        d[wkey] = np.ascontiguousarray(t.reshape(NFC, 128, 3).transpose(1, 0, 2)).astype(BF16)
        z = np.zeros((128, 1), np.float32)
        z[:3, 0] = np.asarray(inputs[bname], np.float32)
        d[bkey] = z
    d["i128b"] = np.eye(128, dtype=BF16)
    d["i128f"] = np.eye(128, dtype=np.float32)
    return d


def _prepare_in_maps(inputs):
    lh = np.asarray(inputs["last_hidden"], np.float32)
    arf = np.asarray(inputs["arousal_feats"], np.float32)
    sep1 = np.asarray(inputs["sep1"]).astype(np.int64)
    sep2 = np.asarray(inputs["sep2"]).astype(np.int64)
    ln_g = np.asarray(inputs["ln_g"], np.float32)
    ln_b = np.asarray(inputs["ln_b"], np.float32)

    shared = _prep_shared(inputs)

    pos = np.arange(S)
    tm = np.where((sep1 > 1)[:, None],
                  (pos[None, :] >= 1) & (pos[None, :] < sep1[:, None]),
                  pos[None, :] == 0)
    am = np.where((sep2 > sep1 + 1)[:, None],
                  (pos[None, :] > sep1[:, None]) & (pos[None, :] < sep2[:, None]),
                  pos[None, :] == 0)
    maskadd = np.where(tm, 0.0, -1e30).astype(np.float32)
    cnt = am.sum(1).astype(np.float32)

    # ragged tile bounds per sample
    n1_all = np.where(sep1 > 1, np.ceil(sep1 / 128), 1).astype(np.int64)
    n1_all = np.maximum(n1_all, 1)
    n2a = np.where(sep2 > sep1 + 1, np.ceil(sep2 / 128), 1).astype(np.int64)
    n2_all = np.maximum(n1_all, np.maximum(n2a, 1))
    # sort samples by size desc, deal round-robin to cores so slot j is
    # similar across cores; per-slot bound = max over cores
    order = np.argsort(-(n2_all * 16 + n1_all), kind="stable")
    perm = np.empty(B, np.int64)
    for c in range(NCORE):
        for j in range(PER):
            perm[c * PER + j] = order[j * NCORE + c]
    n1s = tuple(int(max(n1_all[perm[c * PER + j]] for c in range(NCORE)))
                for j in range(PER))
    n2s = tuple(int(max(max(n2_all[perm[c * PER + j]], n1_all[perm[c * PER + j]])
                        for c in range(NCORE))) for j in range(PER))
    n2s = tuple(max(a, b) for a, b in zip(n1s, n2s))
    # first tile containing any aspect-mask weight (fallback case -> tile 0)
    a0_all = np.where(sep2 > sep1 + 1, sep1 // 128, 0).astype(np.int64)
    a0s = tuple(int(min(a0_all[perm[c * PER + j]] for c in range(NCORE)))
                for j in range(PER))
    a0s = tuple(min(a, b - 1) for a, b in zip(a0s, n2s))

    in_maps = []
    for c in range(NCORE):
        idx = perm[c * PER:(c + 1) * PER]
        m = dict(shared)
        m["xhi"] = lh[idx].astype(BF16)
        m["aspc"] = np.ascontiguousarray(
            am[idx].astype(np.float32).astype(BF16)
            .reshape(PER, NST, 128).transpose(0, 2, 1))
        m["maskr"] = np.ascontiguousarray(
            np.repeat(maskadd[idx][:, None, :], NH, axis=1)).astype(BF16)
        m["clsT"] = np.ascontiguousarray(
            lh[idx, 0, :].T.reshape(NHC, 128, PER)).astype(BF16)
        m["invc"] = np.tile((1.0 / cnt[idx])[None, :], (128, 1)).astype(np.float32)
        m["arof"] = np.ascontiguousarray(arf[idx])
        m["lng"] = np.tile(ln_g[None, :], (PER, 1)).astype(np.float32)
        m["lnb"] = np.tile(ln_b[None, :], (PER, 1)).astype(np.float32)
        in_maps.append(m)
    return in_maps, perm, n1s, n2s, a0s


def _assemble(results, perm):
    va = np.zeros((B, 2), np.float32)
    pol = np.zeros((B, 3), np.float32)
    aro = np.zeros((B, 3), np.float32)
    for c in range(NCORE):
        idx = perm[c * PER:(c + 1) * PER]
        va[idx] = results[c]["va_t"].T
        pol[idx] = results[c]["pol_t"].T
        aro[idx] = results[c]["aro_t"].T
    return (va, pol, aro)


def kernel(**inputs):
    in_maps, perm, n1s, n2s, a0s = _prepare_in_maps(inputs)
    nc = _get_nc(n1s, n2s, a0s)
    res = run_bass_kernel_spmd(nc, in_maps, core_ids=list(range(NCORE)))
    return _assemble(res.results, perm)
